# revision 8
# baseline (speedup 1.0000x reference)
"""Trainium2 Bass kernel for DeltaGradientDescent.

reference math:
    x_n   = x / (||x||_2 + eps)                  per row, x: [64, 4096]
    outer = x_n^T x_n / B                        rank-64, [4096, 4096]
    out   = W @ (I - alpha*outer) - lr*G
          = W - (alpha/B) * (W x_n^T) x_n - lr*G

Sharding: W and G row-sharded across 8 cores (512 rows each); x replicated.
W is shipped host-pre-transposed+packed (pure layout transform) so the PE
never has to transpose it on-chip. Per core, with c1 = alpha/B:
    YT   = x @ W_c^T               (PE, rhs = packed W^T chunks)
    YT_n = diag(1/(||x||+eps)) YT  (left-factor normalization, fused in the
                                    PSUM->SBUF copy)
    PSUM = Y_n @ x_n + (lr/c1) G_c - (1/c1) W_c
           (G via scaled-identity matmul; W via per-chunk matmuls against a
            scaled identity using the packed-W^T slices as stationary)
    out_c = -c1 * PSUM             (single tensor_scalar_mul on DVE)
"""

import numpy as np

import concourse.bass as bass
import concourse.mybir as mybir
import concourse.tile as tile
from concourse import bacc
from concourse.bass_utils import run_bass_kernel_spmd
from concourse.masks import make_identity

F32 = mybir.dt.float32

DIM = 4096
B = 64
NCORES = 8
R = DIM // NCORES  # 512 rows per core
P = 128
NIC = R // P       # 4 row chunks of 128 per core
NJC = DIM // P     # 32 column chunks of 128
NKK = 4            # wtp DMA chunks
JCPK = NJC // NKK  # 8 j-chunks per wtp DMA chunk
LR = 0.001
ALPHA = 0.01
EPS = 1e-8
C1 = ALPHA / B          # 1.5625e-4
S2 = LR / C1            # +6.4    (scale on G inside the PSUM accumulation)
S1 = -1.0 / C1          # -6400.0 (scale on W inside the PSUM accumulation)

_NC_CACHE = {}


def _scaled_identity(nc, ident_ap, value):
    """ident_ap <- value * I (same trick as concourse.masks.make_identity)."""
    nc.gpsimd.memset(ident_ap, 0.0)
    nc.gpsimd.affine_select(
        out=ident_ap,
        in_=ident_ap,
        compare_op=mybir.AluOpType.not_equal,
        fill=float(value),
        base=0,
        pattern=[[-1, ident_ap.shape[0]]],
        channel_multiplier=1,
    )


def _build_kernel(ctx, tc, wtp, g, x, xtp, out):
    nc = tc.nc

    singles = ctx.enter_context(tc.tile_pool(name="singles", bufs=1))
    wpool = ctx.enter_context(tc.tile_pool(name="wpool", bufs=NKK))
    gpool = ctx.enter_context(tc.tile_pool(name="gpool", bufs=3))
    opool = ctx.enter_context(tc.tile_pool(name="opool", bufs=2))
    ypsum = ctx.enter_context(tc.tile_pool(name="ypsum", bufs=1, space="PSUM"))
    zpool = ctx.enter_context(tc.tile_pool(name="zpool", bufs=6, space="PSUM"))

    # ---- W^T packed chunks: wtp_sb[kk][p, c*512 + i] = W_c[i, (kk*8+c)*128 + p]
    wtp_sb = []
    for kk in range(NKK):
        wt_chunk = wpool.tile([P, JCPK * R], F32, tag="wtp_sb")
        nc.sync.dma_start(
            out=wt_chunk, in_=wtp[:, kk * JCPK * R : (kk + 1) * JCPK * R]
        )
        wtp_sb.append(wt_chunk)

    def wt_slice(jc, lo, width):
        """[128, width] slice of W^T chunk jc: partitions = j-local, free = i."""
        return wtp_sb[jc // JCPK][:, (jc % JCPK) * R + lo : (jc % JCPK) * R + lo + width]

    # ---- other inputs ----
    x_sb = singles.tile([B, DIM], F32)
    nc.sync.dma_start(out=x_sb, in_=x[:, :])
    xtp_sb = singles.tile([P, NJC * B], F32)
    nc.sync.dma_start(out=xtp_sb, in_=xtp[:, :])

    ident_g = singles.tile([P, P], F32)
    _scaled_identity(nc, ident_g, S2)
    ident_w = singles.tile([P, P], F32)
    _scaled_identity(nc, ident_w, S1)

    # ---- row norms of x: s = sum(x^2) per row, via bn_stats (tiny scratch) ----
    nstats = DIM // 512
    stats = singles.tile([B, nstats, 6], F32)
    for si in range(nstats):
        nc.vector.bn_stats(out=stats[:, si, :], in_=x_sb[:, si * 512 : (si + 1) * 512])
    mv = singles.tile([B, 2], F32)
    nc.vector.bn_aggr(out=mv, in_=stats)
    # sum(x^2) = DIM * (var + mean^2)
    msq = singles.tile([B, 1], F32)
    nc.scalar.activation(out=msq, in_=mv[:, 0:1], func=mybir.ActivationFunctionType.Square)
    ssum = singles.tile([B, 1], F32)
    nc.vector.tensor_add(ssum, msq, mv[:, 1:2])
    norm = singles.tile([B, 1], F32)
    nc.scalar.activation(
        out=norm, in_=ssum, func=mybir.ActivationFunctionType.Sqrt, scale=float(DIM)
    )
    nc.vector.tensor_scalar_add(norm, norm, EPS)
    rinv = singles.tile([B, 1], F32)
    nc.vector.reciprocal(rinv, norm)
    # x_sb <- x_n (normalized in place)
    nc.vector.tensor_scalar_mul(x_sb, x_sb, rinv)

    # ---- YT = x @ W_c^T : [64, 512] accumulated over 32 j-chunks ----
    yt_ps = ypsum.tile([B, R], F32)
    for jc in range(NJC):
        nc.tensor.matmul(
            yt_ps,
            lhsT=xtp_sb[:, jc * B : (jc + 1) * B],
            rhs=wt_slice(jc, 0, R),
            start=(jc == 0),
            stop=(jc == NJC - 1),
        )
    # fold left-factor normalization into the PSUM->SBUF copy
    yt_sb = singles.tile([B, R], F32)
    nc.vector.tensor_scalar_mul(yt_sb, yt_ps, rinv)

    # ---- Z + combine, streamed over ic output chunks of [128, 4096] ----
    QW = DIM // 512  # 8 x 512-wide tiles per row chunk
    for ic in range(NIC):
        g_t = gpool.tile([P, DIM], F32, tag="g_t")
        nc.sync.dma_start(out=g_t, in_=g[ic * P : (ic + 1) * P, :])
        o_t = opool.tile([P, DIM], F32, tag="o_t")
        for q in range(QW):
            z_ps = zpool.tile([P, 512], F32, tag="z_ps")
            # + (lr/c1) * G   (can run before Y^T is ready)
            nc.tensor.matmul(
                z_ps,
                lhsT=ident_g,
                rhs=g_t[:, q * 512 : (q + 1) * 512],
                start=True,
                stop=False,
            )
            # - (1/c1) * W from the packed-transpose slices
            for t in range(4):
                jc = 4 * q + t
                nc.tensor.matmul(
                    z_ps[:, t * P : (t + 1) * P],
                    lhsT=wt_slice(jc, ic * P, P),
                    rhs=ident_w,
                    start=False,
                    stop=False,
                )
            # rank-64 product: Z_tile = Y_n @ x_n  (last; waits on yt_sb)
            nc.tensor.matmul(
                z_ps,
                lhsT=yt_sb[:, ic * P : (ic + 1) * P],
                rhs=x_sb[:, q * 512 : (q + 1) * 512],
                start=False,
                stop=True,
            )
            nc.vector.tensor_scalar_mul(o_t[:, q * 512 : (q + 1) * 512], z_ps, -C1)
        nc.scalar.dma_start(out=out[ic * P : (ic + 1) * P, :], in_=o_t)


def _get_nc(reps=1):
    key = ("nc", reps)
    if key in _NC_CACHE:
        return _NC_CACHE[key]
    from contextlib import ExitStack

    nc = bacc.Bacc(None, target_bir_lowering=False, debug=False)
    wtp = nc.declare_dram_parameter("wtp", [P, NJC * R], F32, isOutput=False).ap()
    g = nc.declare_dram_parameter("g", [R, DIM], F32, isOutput=False).ap()
    x = nc.declare_dram_parameter("x", [B, DIM], F32, isOutput=False).ap()
    xtp = nc.declare_dram_parameter("xtp", [P, NJC * B], F32, isOutput=False).ap()
    out = nc.declare_dram_parameter("out", [R, DIM], F32, isOutput=True).ap()
    with tile.TileContext(nc) as tc:
        for _ in range(reps):
            with ExitStack() as ctx:
                _build_kernel(ctx, tc, wtp, g, x, xtp, out)
    nc.finalize()
    _NC_CACHE[key] = nc
    return nc


def _make_in_maps(weight, input_x, grad):
    weight = np.asarray(weight, dtype=np.float32)
    grad = np.asarray(grad, dtype=np.float32)
    x = np.ascontiguousarray(np.asarray(input_x, dtype=np.float32))
    # host-side layout packing of x^T into 128-partition chunk form:
    # xtp[p, jc*64 + b] = x[b, jc*128 + p]
    xtp = np.ascontiguousarray(
        x.T.reshape(NJC, P, B).transpose(1, 0, 2).reshape(P, NJC * B)
    )
    in_maps = []
    for c in range(NCORES):
        w_c = weight[c * R : (c + 1) * R]
        # packed transpose: wtp[p, jc*512 + i] = w_c[i, jc*128 + p]
        wtp = np.ascontiguousarray(
            w_c.T.reshape(NJC, P, R).transpose(1, 0, 2).reshape(P, NJC * R)
        )
        in_maps.append(
            {
                "wtp": wtp,
                "g": np.ascontiguousarray(grad[c * R : (c + 1) * R]),
                "x": x,
                "xtp": xtp,
            }
        )
    return in_maps


def run(weight, input_x, grad, trace=False, **kwargs):
    """Run the SPMD kernel; returns (full output, BassKernelResults)."""
    nc = _get_nc()
    in_maps = _make_in_maps(weight, input_x, grad)
    res = run_bass_kernel_spmd(nc, in_maps, list(range(NCORES)), trace=trace, **kwargs)
    out = np.concatenate(
        [np.asarray(res.results[c]["out"], dtype=np.float32) for c in range(NCORES)],
        axis=0,
    )
    return out, res


def kernel(weight, input_x, grad):
    out, _ = run(weight, input_x, grad, trace=False)
    return out


# revision 9
# speedup vs baseline: 1.8744x; 1.8744x over previous
"""Trainium2 Bass kernel for DeltaGradientDescent.

reference math:
    x_n   = x / (||x||_2 + eps)                  per row, x: [64, 4096]
    outer = x_n^T x_n / B                        rank-64, [4096, 4096]
    out   = W @ (I - alpha*outer) - lr*G
          = W - (alpha/B) * (W x_n^T) x_n - lr*G

Sharding: W and G row-sharded across 8 cores (512 rows each); x replicated.

Per core, with c1 = alpha/B:
    YT   = x @ W_c^T          (PE; W^T arrives host-pre-transposed in bf16 —
                               it only feeds the c1-scaled low-rank term, so
                               bf16 rounding is ~2^-9 * c1 in the output)
    YT_n = diag(1/(||x||+eps)) YT   (normalization of the left factor, fused
                                     into the PSUM->SBUF copy, cast to bf16)
    Z    = YT_n^T @ x_n       (PE, rank-64 product, PSUM f32)
    out  = (-c1 * Z + W_c) + (-lr) * G_c   (two fused scalar_tensor_tensor
                                            passes on DVE, all f32 — the
                                            full-magnitude terms stay exact)
"""

import numpy as np
import ml_dtypes

import concourse.bass as bass
import concourse.mybir as mybir
import concourse.tile as tile
from concourse import bacc
from concourse.bass_utils import run_bass_kernel_spmd

F32 = mybir.dt.float32
BF16 = mybir.dt.bfloat16
NP_BF16 = ml_dtypes.bfloat16

DIM = 4096
B = 64
NCORES = 8
R = DIM // NCORES  # 512 rows per core
P = 128
NIC = R // P       # 4 row chunks of 128 per core
NJC = DIM // P     # 32 column chunks of 128
NKK = 4            # packed-W^T DMA chunks
JCPK = NJC // NKK  # 8 j-chunks per packed-W^T DMA chunk
LR = 0.001
ALPHA = 0.01
EPS = 1e-8
C1 = ALPHA / B     # 1.5625e-4

_NC_CACHE = {}


def _build_kernel(ctx, tc, w, wtb, g, x, xtb, out):
    nc = tc.nc

    singles = ctx.enter_context(tc.tile_pool(name="singles", bufs=1))
    wtpool = ctx.enter_context(tc.tile_pool(name="wtpool", bufs=NKK))
    wpool = ctx.enter_context(tc.tile_pool(name="wpool", bufs=NIC))
    gpool = ctx.enter_context(tc.tile_pool(name="gpool", bufs=3))
    opool = ctx.enter_context(tc.tile_pool(name="opool", bufs=3))
    ypsum = ctx.enter_context(tc.tile_pool(name="ypsum", bufs=1, space="PSUM"))
    zpool = ctx.enter_context(tc.tile_pool(name="zpool", bufs=6, space="PSUM"))

    # ---- bf16 W^T packed chunks (Y^T path only):
    #      wtb_sb[kk][p, c*512 + i] = W_c[i, (kk*8+c)*128 + p]
    wtb_sb = []
    for kk in range(NKK):
        t = wtpool.tile([P, JCPK * R], BF16, tag="wtb_sb")
        nc.sync.dma_start(out=t, in_=wtb[:, kk * JCPK * R : (kk + 1) * JCPK * R])
        wtb_sb.append(t)

    # ---- x (f32, for norms) and packed x^T (bf16, Y^T stationary) ----
    x_sb = singles.tile([B, DIM], F32)
    nc.sync.dma_start(out=x_sb, in_=x[:, :])
    xtb_sb = singles.tile([P, NJC * B], BF16)
    nc.sync.dma_start(out=xtb_sb, in_=xtb[:, :])

    # ---- W natural chunks (combine path, f32, full magnitude) ----
    w_sb = []
    for ic in range(NIC):
        t = wpool.tile([P, DIM], F32, tag="w_sb")
        nc.sync.dma_start(out=t, in_=w[ic * P : (ic + 1) * P, :])
        w_sb.append(t)

    # ---- row norms of x: s = sum(x^2) per row, via bn_stats (tiny scratch) ----
    nstats = DIM // 512
    stats = singles.tile([B, nstats, 6], F32)
    for si in range(nstats):
        nc.vector.bn_stats(out=stats[:, si, :], in_=x_sb[:, si * 512 : (si + 1) * 512])
    mv = singles.tile([B, 2], F32)
    nc.vector.bn_aggr(out=mv, in_=stats)
    # sum(x^2) = DIM * (var + mean^2)
    msq = singles.tile([B, 1], F32)
    nc.scalar.activation(out=msq, in_=mv[:, 0:1], func=mybir.ActivationFunctionType.Square)
    ssum = singles.tile([B, 1], F32)
    nc.vector.tensor_add(ssum, msq, mv[:, 1:2])
    norm = singles.tile([B, 1], F32)
    nc.scalar.activation(
        out=norm, in_=ssum, func=mybir.ActivationFunctionType.Sqrt, scale=float(DIM)
    )
    nc.vector.tensor_scalar_add(norm, norm, EPS)
    rinv = singles.tile([B, 1], F32)
    nc.vector.reciprocal(rinv, norm)
    # normalized x in bf16 (rhs of the rank-64 product; c1-scaled path)
    xn_sb = singles.tile([B, DIM], BF16)
    nc.vector.tensor_scalar_mul(xn_sb, x_sb, rinv)

    # ---- YT = x @ W_c^T : [64, 512] accumulated over 32 j-chunks ----
    yt_ps = ypsum.tile([B, R], F32)
    for jc in range(NJC):
        nc.tensor.matmul(
            yt_ps,
            lhsT=xtb_sb[:, jc * B : (jc + 1) * B],
            rhs=wtb_sb[jc // JCPK][:, (jc % JCPK) * R : (jc % JCPK + 1) * R],
            start=(jc == 0),
            stop=(jc == NJC - 1),
        )
    # normalization of the left factor, fused into the PSUM->SBUF copy
    yt_sb = singles.tile([B, R], BF16)
    nc.vector.tensor_scalar_mul(yt_sb, yt_ps, rinv)

    # ---- Z + combine, streamed over (ic, half) output chunks of [128, 2048] ----
    HALF = 2
    QW = 4
    for ic in range(NIC):
        for h in range(HALF):
            g_t = gpool.tile([P, 2048], F32, tag="g_t")
            nc.sync.dma_start(
                out=g_t, in_=g[ic * P : (ic + 1) * P, h * 2048 : (h + 1) * 2048]
            )
            o_t = opool.tile([P, 2048], F32, tag="o_t")
            for q in range(QW):
                jb = h * QW + q
                z_ps = zpool.tile([P, 512], F32, tag="z_ps")
                nc.tensor.matmul(
                    z_ps,
                    lhsT=yt_sb[:, ic * P : (ic + 1) * P],
                    rhs=xn_sb[:, jb * 512 : (jb + 1) * 512],
                    start=True,
                    stop=True,
                )
                # out = -c1*Z + W      (one fused DVE op, PSUM source)
                nc.vector.scalar_tensor_tensor(
                    out=o_t[:, q * 512 : (q + 1) * 512],
                    in0=z_ps,
                    scalar=-C1,
                    in1=w_sb[ic][:, jb * 512 : (jb + 1) * 512],
                    op0=mybir.AluOpType.mult,
                    op1=mybir.AluOpType.add,
                )
                # out += -lr*G         (second fused DVE op, in place)
                nc.vector.scalar_tensor_tensor(
                    out=o_t[:, q * 512 : (q + 1) * 512],
                    in0=g_t[:, q * 512 : (q + 1) * 512],
                    scalar=-LR,
                    in1=o_t[:, q * 512 : (q + 1) * 512],
                    op0=mybir.AluOpType.mult,
                    op1=mybir.AluOpType.add,
                )
            nc.scalar.dma_start(
                out=out[ic * P : (ic + 1) * P, h * 2048 : (h + 1) * 2048], in_=o_t
            )


def _get_nc(reps=1):
    key = ("nc", reps)
    if key in _NC_CACHE:
        return _NC_CACHE[key]
    from contextlib import ExitStack

    nc = bacc.Bacc(None, target_bir_lowering=False, debug=False)
    w = nc.declare_dram_parameter("w", [R, DIM], F32, isOutput=False).ap()
    wtb = nc.declare_dram_parameter("wtb", [P, NJC * R], BF16, isOutput=False).ap()
    g = nc.declare_dram_parameter("g", [R, DIM], F32, isOutput=False).ap()
    x = nc.declare_dram_parameter("x", [B, DIM], F32, isOutput=False).ap()
    xtb = nc.declare_dram_parameter("xtb", [P, NJC * B], BF16, isOutput=False).ap()
    out = nc.declare_dram_parameter("out", [R, DIM], F32, isOutput=True).ap()
    with tile.TileContext(nc) as tc:
        for _ in range(reps):
            with ExitStack() as ctx:
                _build_kernel(ctx, tc, w, wtb, g, x, xtb, out)
    nc.finalize()
    _NC_CACHE[key] = nc
    return nc


def _make_in_maps(weight, input_x, grad):
    weight = np.asarray(weight, dtype=np.float32)
    grad = np.asarray(grad, dtype=np.float32)
    x = np.ascontiguousarray(np.asarray(input_x, dtype=np.float32))
    # host-side layout packing of x^T into 128-partition chunk form (bf16):
    # xtb[p, jc*64 + b] = x[b, jc*128 + p]
    xtb = np.ascontiguousarray(
        x.T.reshape(NJC, P, B).transpose(1, 0, 2).reshape(P, NJC * B).astype(NP_BF16)
    )
    in_maps = []
    for c in range(NCORES):
        w_c = weight[c * R : (c + 1) * R]
        # packed transpose (bf16): wtb[p, jc*512 + i] = w_c[i, jc*128 + p]
        wtb = np.ascontiguousarray(
            w_c.T.reshape(NJC, P, R).transpose(1, 0, 2).reshape(P, NJC * R)
            .astype(NP_BF16)
        )
        in_maps.append(
            {
                "w": np.ascontiguousarray(w_c),
                "wtb": wtb,
                "g": np.ascontiguousarray(grad[c * R : (c + 1) * R]),
                "x": x,
                "xtb": xtb,
            }
        )
    return in_maps


def run(weight, input_x, grad, trace=False, **kwargs):
    """Run the SPMD kernel; returns (full output, BassKernelResults)."""
    nc = _get_nc()
    in_maps = _make_in_maps(weight, input_x, grad)
    res = run_bass_kernel_spmd(nc, in_maps, list(range(NCORES)), trace=trace, **kwargs)
    out = np.concatenate(
        [np.asarray(res.results[c]["out"], dtype=np.float32) for c in range(NCORES)],
        axis=0,
    )
    return out, res


def kernel(weight, input_x, grad):
    out, _ = run(weight, input_x, grad, trace=False)
    return out


# revision 15
# speedup vs baseline: 2.0369x; 1.0867x over previous
"""Trainium2 Bass kernel for DeltaGradientDescent.

reference math:
    x_n   = x / (||x||_2 + eps)                  per row, x: [64, 4096]
    outer = x_n^T x_n / B                        rank-64, [4096, 4096]
    out   = W @ (I - alpha*outer) - lr*G
          = W - (alpha/B) * (W x_n^T) x_n - lr*G

Sharding: W and G row-sharded across 8 cores (512 rows each); x replicated.

Per core, with c1 = alpha/B:
    YT   = x @ W_c^T          (PE; W^T arrives host-pre-transposed in bf16 —
                               it only feeds the c1-scaled low-rank term, so
                               bf16 rounding is ~2^-9 * c1 in the output)
    YT_n = diag(1/(||x||+eps)) YT   (normalization of the left factor, fused
                                     into the PSUM->SBUF copy, cast to bf16)
    Z    = YT_n^T @ x_n       (PE, rank-64 product, PSUM f32)
    out  = (-c1 * Z + W_c) + (-lr) * G_c   (two fused scalar_tensor_tensor
                                            passes on DVE, all f32 — the
                                            full-magnitude terms stay exact)
"""

import numpy as np
import ml_dtypes

import concourse.bass as bass
import concourse.mybir as mybir
import concourse.tile as tile
from concourse import bacc
from concourse.bass_utils import run_bass_kernel_spmd

F32 = mybir.dt.float32
BF16 = mybir.dt.bfloat16
NP_BF16 = ml_dtypes.bfloat16

DIM = 4096
B = 64
NCORES = 8
R = DIM // NCORES  # 512 rows per core
P = 128
NIC = R // P       # 4 row chunks of 128 per core
NJC = DIM // P     # 32 column chunks of 128
NKK = 4            # packed-W^T DMA chunks
JCPK = NJC // NKK  # 8 j-chunks per packed-W^T DMA chunk
LR = 0.001
ALPHA = 0.01
EPS = 1e-8
C1 = ALPHA / B     # 1.5625e-4

_NC_CACHE = {}


def _build_kernel(ctx, tc, w, wtb, g, x, xtb, out):
    nc = tc.nc

    singles = ctx.enter_context(tc.tile_pool(name="singles", bufs=1))
    wtpool = ctx.enter_context(tc.tile_pool(name="wtpool", bufs=NKK))
    wpool = ctx.enter_context(tc.tile_pool(name="wpool", bufs=NIC))
    gpool = ctx.enter_context(tc.tile_pool(name="gpool", bufs=NIC))
    opool = ctx.enter_context(tc.tile_pool(name="opool", bufs=3))
    ypsum = ctx.enter_context(tc.tile_pool(name="ypsum", bufs=1, space="PSUM"))
    zpool = ctx.enter_context(tc.tile_pool(name="zpool", bufs=6, space="PSUM"))

    # ---- bf16 W^T packed chunks (Y^T path only):
    #      wtb_sb[kk][p, c*512 + i] = W_c[i, (kk*8+c)*128 + p]
    wtb_sb = []
    for kk in range(NKK):
        t = wtpool.tile([P, JCPK * R], BF16, tag="wtb_sb")
        nc.sync.dma_start(out=t, in_=wtb[:, kk * JCPK * R : (kk + 1) * JCPK * R])
        wtb_sb.append(t)

    # ---- x (f32, for norms) and packed x^T (bf16, Y^T stationary) ----
    x_sb = singles.tile([B, DIM], F32)
    nc.sync.dma_start(out=x_sb, in_=x[:, :])
    xtb_sb = singles.tile([P, NJC * B], BF16)
    nc.sync.dma_start(out=xtb_sb, in_=xtb[:, :])

    # ---- W natural (f32, full magnitude) + G (bf16, lr-scaled term),
    #      interleaved per row-chunk so the combine pipeline unblocks early
    w_sb, g_sb = [], []
    for ic in range(NIC):
        t = wpool.tile([P, DIM], F32, tag="w_sb")
        nc.sync.dma_start(out=t, in_=w[ic * P : (ic + 1) * P, :])
        w_sb.append(t)
        tg = gpool.tile([P, DIM], BF16, tag="g_sb")
        nc.sync.dma_start(out=tg, in_=g[ic * P : (ic + 1) * P, :])
        g_sb.append(tg)

    # ---- row norms of x: s = sum(x^2) per row, via bn_stats (tiny scratch) ----
    nstats = DIM // 512
    stats = singles.tile([B, nstats, 6], F32)
    for si in range(nstats):
        nc.vector.bn_stats(out=stats[:, si, :], in_=x_sb[:, si * 512 : (si + 1) * 512])
    mv = singles.tile([B, 2], F32)
    nc.vector.bn_aggr(out=mv, in_=stats)
    # sum(x^2) = DIM * (var + mean^2)
    msq = singles.tile([B, 1], F32)
    nc.scalar.activation(out=msq, in_=mv[:, 0:1], func=mybir.ActivationFunctionType.Square)
    ssum = singles.tile([B, 1], F32)
    nc.vector.tensor_add(ssum, msq, mv[:, 1:2])
    norm = singles.tile([B, 1], F32)
    nc.scalar.activation(
        out=norm, in_=ssum, func=mybir.ActivationFunctionType.Sqrt, scale=float(DIM)
    )
    nc.vector.tensor_scalar_add(norm, norm, EPS)
    rinv = singles.tile([B, 1], F32)
    nc.vector.reciprocal(rinv, norm)
    # normalized x in bf16 (rhs of the rank-64 product; c1-scaled path)
    xn_sb = singles.tile([B, DIM], BF16)
    nc.vector.tensor_scalar_mul(xn_sb, x_sb, rinv)

    # ---- YT = x @ W_c^T : [64, 512] accumulated over 32 j-chunks ----
    yt_ps = ypsum.tile([B, R], F32)
    for jc in range(NJC):
        nc.tensor.matmul(
            yt_ps,
            lhsT=xtb_sb[:, jc * B : (jc + 1) * B],
            rhs=wtb_sb[jc // JCPK][:, (jc % JCPK) * R : (jc % JCPK + 1) * R],
            start=(jc == 0),
            stop=(jc == NJC - 1),
        )
    # normalization of the left factor, fused into the PSUM->SBUF copy
    yt_sb = singles.tile([B, R], BF16)
    nc.vector.tensor_scalar_mul(yt_sb, yt_ps, rinv)

    # ---- Z + combine, streamed over (ic, half) output chunks of [128, 2048] ----
    HALF = 2
    QW = 4
    for ic in range(NIC):
        for h in range(HALF):
            o_t = opool.tile([P, 2048], F32, tag="o_t")
            for q in range(QW):
                jb = h * QW + q
                z_ps = zpool.tile([P, 512], F32, tag="z_ps")
                nc.tensor.matmul(
                    z_ps,
                    lhsT=yt_sb[:, ic * P : (ic + 1) * P],
                    rhs=xn_sb[:, jb * 512 : (jb + 1) * 512],
                    start=True,
                    stop=True,
                )
                # out = -c1*Z + W      (one fused DVE op, PSUM source)
                nc.vector.scalar_tensor_tensor(
                    out=o_t[:, q * 512 : (q + 1) * 512],
                    in0=z_ps,
                    scalar=-C1,
                    in1=w_sb[ic][:, jb * 512 : (jb + 1) * 512],
                    op0=mybir.AluOpType.mult,
                    op1=mybir.AluOpType.add,
                )
                # out += -lr*G         (second fused DVE op, in place)
                nc.vector.scalar_tensor_tensor(
                    out=o_t[:, q * 512 : (q + 1) * 512],
                    in0=g_sb[ic][:, jb * 512 : (jb + 1) * 512],
                    scalar=-LR,
                    in1=o_t[:, q * 512 : (q + 1) * 512],
                    op0=mybir.AluOpType.mult,
                    op1=mybir.AluOpType.add,
                )
            nc.scalar.dma_start(
                out=out[ic * P : (ic + 1) * P, h * 2048 : (h + 1) * 2048], in_=o_t
            )


def _get_nc(reps=1):
    key = ("nc", reps)
    if key in _NC_CACHE:
        return _NC_CACHE[key]
    from contextlib import ExitStack

    nc = bacc.Bacc(None, target_bir_lowering=False, debug=False)
    w = nc.declare_dram_parameter("w", [R, DIM], F32, isOutput=False).ap()
    wtb = nc.declare_dram_parameter("wtb", [P, NJC * R], BF16, isOutput=False).ap()
    g = nc.declare_dram_parameter("g", [R, DIM], BF16, isOutput=False).ap()
    x = nc.declare_dram_parameter("x", [B, DIM], F32, isOutput=False).ap()
    xtb = nc.declare_dram_parameter("xtb", [P, NJC * B], BF16, isOutput=False).ap()
    out = nc.declare_dram_parameter("out", [R, DIM], F32, isOutput=True).ap()
    with tile.TileContext(nc) as tc:
        for _ in range(reps):
            with ExitStack() as ctx:
                _build_kernel(ctx, tc, w, wtb, g, x, xtb, out)
    nc.finalize()
    _NC_CACHE[key] = nc
    return nc


def _make_in_maps(weight, input_x, grad):
    weight = np.asarray(weight, dtype=np.float32)
    grad = np.asarray(grad, dtype=np.float32)
    x = np.ascontiguousarray(np.asarray(input_x, dtype=np.float32))
    # host-side layout packing of x^T into 128-partition chunk form (bf16):
    # xtb[p, jc*64 + b] = x[b, jc*128 + p]
    xtb = np.ascontiguousarray(
        x.T.reshape(NJC, P, B).transpose(1, 0, 2).reshape(P, NJC * B).astype(NP_BF16)
    )
    in_maps = []
    for c in range(NCORES):
        w_c = weight[c * R : (c + 1) * R]
        # packed transpose (bf16): wtb[p, jc*512 + i] = w_c[i, jc*128 + p]
        wtb = np.ascontiguousarray(
            w_c.T.reshape(NJC, P, R).transpose(1, 0, 2).reshape(P, NJC * R)
            .astype(NP_BF16)
        )
        in_maps.append(
            {
                "w": np.ascontiguousarray(w_c),
                "wtb": wtb,
                "g": np.ascontiguousarray(grad[c * R : (c + 1) * R].astype(NP_BF16)),
                "x": x,
                "xtb": xtb,
            }
        )
    return in_maps


def run(weight, input_x, grad, trace=False, **kwargs):
    """Run the SPMD kernel; returns (full output, BassKernelResults)."""
    nc = _get_nc()
    in_maps = _make_in_maps(weight, input_x, grad)
    res = run_bass_kernel_spmd(nc, in_maps, list(range(NCORES)), trace=trace, **kwargs)
    out = np.concatenate(
        [np.asarray(res.results[c]["out"], dtype=np.float32) for c in range(NCORES)],
        axis=0,
    )
    return out, res


def kernel(weight, input_x, grad):
    out, _ = run(weight, input_x, grad, trace=False)
    return out


# revision 23
# speedup vs baseline: 2.1382x; 1.0498x over previous
"""Trainium2 Bass kernel for DeltaGradientDescent.

reference math:
    x_n   = x / (||x||_2 + eps)                  per row, x: [64, 4096]
    outer = x_n^T x_n / B                        rank-64, [4096, 4096]
    out   = W @ (I - alpha*outer) - lr*G
          = W - (alpha/B) * (W x_n^T) x_n - lr*G

Sharding: W and G row-sharded across 8 cores (512 rows each); x replicated.

Per core, with c1 = alpha/B:
    YT   = x @ W_c^T          (PE; W^T arrives host-pre-transposed in bf16 —
                               it only feeds the c1-scaled low-rank term, so
                               bf16 rounding is ~2^-9 * c1 in the output)
    YT_n = diag(1/(||x||+eps)) YT   (normalization of the left factor, fused
                                     into the PSUM->SBUF copy, cast to bf16)
    Z    = YT_n^T @ x_n       (PE, rank-64 product, PSUM f32)
    out  = (-c1 * Z + W_c) + (-lr) * G_c   (two fused scalar_tensor_tensor
                                            passes on DVE, all f32 — the
                                            full-magnitude terms stay exact)
"""

import numpy as np
import ml_dtypes

import concourse.bass as bass
import concourse.mybir as mybir
import concourse.tile as tile
from concourse import bacc
from concourse.bass_utils import run_bass_kernel_spmd

F32 = mybir.dt.float32
BF16 = mybir.dt.bfloat16
NP_BF16 = ml_dtypes.bfloat16

DIM = 4096
B = 64
NCORES = 8
R = DIM // NCORES  # 512 rows per core
P = 128
NIC = R // P       # 4 row chunks of 128 per core
NJC = DIM // P     # 32 column chunks of 128
NKK = 4            # packed-W^T DMA chunks
JCPK = NJC // NKK  # 8 j-chunks per packed-W^T DMA chunk
LR = 0.001
ALPHA = 0.01
EPS = 1e-8
C1 = ALPHA / B     # 1.5625e-4

_NC_CACHE = {}


def _build_kernel(ctx, tc, w, wtb, g, x, xtb, out):
    nc = tc.nc

    singles = ctx.enter_context(tc.tile_pool(name="singles", bufs=1))
    wtpool = ctx.enter_context(tc.tile_pool(name="wtpool", bufs=NKK))
    wpool = ctx.enter_context(tc.tile_pool(name="wpool", bufs=NIC))
    gpool = ctx.enter_context(tc.tile_pool(name="gpool", bufs=NIC))
    opool = ctx.enter_context(tc.tile_pool(name="opool", bufs=3))
    ypsum = ctx.enter_context(tc.tile_pool(name="ypsum", bufs=1, space="PSUM"))
    zpool = ctx.enter_context(tc.tile_pool(name="zpool", bufs=6, space="PSUM"))

    # ---- x (f32, for norms) and packed x^T (bf16, Y^T stationary) first:
    #      they gate the normalization chain and the Y^T matmuls
    x_sb = singles.tile([B, DIM], F32)
    nc.sync.dma_start(out=x_sb, in_=x[:, :])
    xtb_sb = singles.tile([P, NJC * B], BF16)
    nc.sync.dma_start(out=xtb_sb, in_=xtb[:, :])

    # ---- bf16 W^T packed chunks (Y^T path only):
    #      wtb_sb[kk][p, c*512 + i] = W_c[i, (kk*8+c)*128 + p]
    wtb_sb = []
    for kk in range(NKK):
        t = wtpool.tile([P, JCPK * R], BF16, tag="wtb_sb")
        nc.sync.dma_start(out=t, in_=wtb[:, kk * JCPK * R : (kk + 1) * JCPK * R])
        wtb_sb.append(t)

    # ---- W natural (f32, full magnitude) + G (bf16, lr-scaled term),
    #      interleaved per row-chunk; G first in each pair (PE consumes it
    #      earlier than DVE consumes W) ----
    w_sb, g_sb = [], []
    for ic in range(NIC):
        tg = gpool.tile([P, DIM], BF16, tag="g_sb")
        nc.sync.dma_start(out=tg, in_=g[ic * P : (ic + 1) * P, :])
        g_sb.append(tg)
        t = wpool.tile([P, DIM], F32, tag="w_sb")
        nc.sync.dma_start(out=t, in_=w[ic * P : (ic + 1) * P, :])
        w_sb.append(t)

    # ---- bf16 scaled identity for the G term: (lr/c1) * I ----
    ident_g = singles.tile([P, P], BF16)
    nc.gpsimd.memset(ident_g, 0.0)
    nc.gpsimd.affine_select(
        out=ident_g,
        in_=ident_g,
        compare_op=mybir.AluOpType.not_equal,
        fill=float(LR / C1),
        base=0,
        pattern=[[-1, P]],
        channel_multiplier=1,
    )

    # ---- row norms of x: s = sum(x^2) per row, via bn_stats (tiny scratch) ----
    nstats = DIM // 512
    stats = singles.tile([B, nstats, 6], F32)
    for si in range(nstats):
        nc.vector.bn_stats(out=stats[:, si, :], in_=x_sb[:, si * 512 : (si + 1) * 512])
    mv = singles.tile([B, 2], F32)
    nc.vector.bn_aggr(out=mv, in_=stats)
    # sum(x^2) = DIM * (var + mean^2)
    msq = singles.tile([B, 1], F32)
    nc.scalar.activation(out=msq, in_=mv[:, 0:1], func=mybir.ActivationFunctionType.Square)
    ssum = singles.tile([B, 1], F32)
    nc.vector.tensor_add(ssum, msq, mv[:, 1:2])
    norm = singles.tile([B, 1], F32)
    nc.scalar.activation(
        out=norm, in_=ssum, func=mybir.ActivationFunctionType.Sqrt, scale=float(DIM)
    )
    nc.vector.tensor_scalar_add(norm, norm, EPS)
    rinv = singles.tile([B, 1], F32)
    nc.vector.reciprocal(rinv, norm)
    # normalized x in bf16 (rhs of the rank-64 product; c1-scaled path)
    xn_sb = singles.tile([B, DIM], BF16)
    nc.vector.tensor_scalar_mul(xn_sb, x_sb, rinv)

    # ---- YT = x @ W_c^T : [64, 512] accumulated over 32 j-chunks ----
    yt_ps = ypsum.tile([B, R], F32)
    for jc in range(NJC):
        nc.tensor.matmul(
            yt_ps,
            lhsT=xtb_sb[:, jc * B : (jc + 1) * B],
            rhs=wtb_sb[jc // JCPK][:, (jc % JCPK) * R : (jc % JCPK + 1) * R],
            start=(jc == 0),
            stop=(jc == NJC - 1),
        )
    # normalization of the left factor, fused into the PSUM->SBUF copy
    yt_sb = singles.tile([B, R], BF16)
    nc.vector.tensor_scalar_mul(yt_sb, yt_ps, rinv)

    # ---- Z + combine, streamed over (ic, half) output chunks of [128, 2048] ----
    HALF = 2
    QW = 4
    for ic in range(NIC):
        for h in range(HALF):
            o_t = opool.tile([P, 2048], F32, tag="o_t")
            zs = []
            # + (lr/c1)*G, grouped: one ident_g LDWEIGHTS for all 4 tiles
            for q in range(QW):
                jb = h * QW + q
                z_ps = zpool.tile([P, 512], F32, tag="z_ps")
                nc.tensor.matmul(
                    z_ps,
                    lhsT=ident_g,
                    rhs=g_sb[ic][:, jb * 512 : (jb + 1) * 512],
                    start=True,
                    stop=False,
                )
                zs.append(z_ps)
            # + Y_n @ x_n, grouped: one yt LDWEIGHTS for all 4 tiles
            for q in range(QW):
                jb = h * QW + q
                nc.tensor.matmul(
                    zs[q],
                    lhsT=yt_sb[:, ic * P : (ic + 1) * P],
                    rhs=xn_sb[:, jb * 512 : (jb + 1) * 512],
                    start=False,
                    stop=True,
                )
            # out = -c1*Z + W      (one fused DVE op per tile, PSUM source)
            for q in range(QW):
                jb = h * QW + q
                nc.vector.scalar_tensor_tensor(
                    out=o_t[:, q * 512 : (q + 1) * 512],
                    in0=zs[q],
                    scalar=-C1,
                    in1=w_sb[ic][:, jb * 512 : (jb + 1) * 512],
                    op0=mybir.AluOpType.mult,
                    op1=mybir.AluOpType.add,
                )
            nc.scalar.dma_start(
                out=out[ic * P : (ic + 1) * P, h * 2048 : (h + 1) * 2048], in_=o_t
            )


def _get_nc(reps=1):
    key = ("nc", reps)
    if key in _NC_CACHE:
        return _NC_CACHE[key]
    from contextlib import ExitStack

    nc = bacc.Bacc(None, target_bir_lowering=False, debug=False)
    w = nc.declare_dram_parameter("w", [R, DIM], F32, isOutput=False).ap()
    wtb = nc.declare_dram_parameter("wtb", [P, NJC * R], BF16, isOutput=False).ap()
    g = nc.declare_dram_parameter("g", [R, DIM], BF16, isOutput=False).ap()
    x = nc.declare_dram_parameter("x", [B, DIM], F32, isOutput=False).ap()
    xtb = nc.declare_dram_parameter("xtb", [P, NJC * B], BF16, isOutput=False).ap()
    out = nc.declare_dram_parameter("out", [R, DIM], F32, isOutput=True).ap()
    with tile.TileContext(nc) as tc:
        for _ in range(reps):
            with ExitStack() as ctx:
                _build_kernel(ctx, tc, w, wtb, g, x, xtb, out)
    nc.finalize()
    _NC_CACHE[key] = nc
    return nc


def _make_in_maps(weight, input_x, grad):
    weight = np.asarray(weight, dtype=np.float32)
    grad = np.asarray(grad, dtype=np.float32)
    x = np.ascontiguousarray(np.asarray(input_x, dtype=np.float32))
    # host-side layout packing of x^T into 128-partition chunk form (bf16):
    # xtb[p, jc*64 + b] = x[b, jc*128 + p]
    xtb = np.ascontiguousarray(
        x.T.reshape(NJC, P, B).transpose(1, 0, 2).reshape(P, NJC * B).astype(NP_BF16)
    )
    in_maps = []
    for c in range(NCORES):
        w_c = weight[c * R : (c + 1) * R]
        # packed transpose (bf16): wtb[p, jc*512 + i] = w_c[i, jc*128 + p]
        wtb = np.ascontiguousarray(
            w_c.T.reshape(NJC, P, R).transpose(1, 0, 2).reshape(P, NJC * R)
            .astype(NP_BF16)
        )
        in_maps.append(
            {
                "w": np.ascontiguousarray(w_c),
                "wtb": wtb,
                "g": np.ascontiguousarray(grad[c * R : (c + 1) * R].astype(NP_BF16)),
                "x": x,
                "xtb": xtb,
            }
        )
    return in_maps


def run(weight, input_x, grad, trace=False, **kwargs):
    """Run the SPMD kernel; returns (full output, BassKernelResults)."""
    nc = _get_nc()
    in_maps = _make_in_maps(weight, input_x, grad)
    res = run_bass_kernel_spmd(nc, in_maps, list(range(NCORES)), trace=trace, **kwargs)
    out = np.concatenate(
        [np.asarray(res.results[c]["out"], dtype=np.float32) for c in range(NCORES)],
        axis=0,
    )
    return out, res


def kernel(weight, input_x, grad):
    out, _ = run(weight, input_x, grad, trace=False)
    return out
